# revision 34
# baseline (speedup 1.0000x reference)
"""3D Haar DWT (single level) on Trainium2, data-parallel over 8 NeuronCores.

Input  x: [2, 32, 32, 128, 128] f32  (B, C, D, H, W)
Output (LLL [2,32,16,64,64], H_all [2,224,16,64,64])  -- same pytree as the
reference: H_all = concat([LLH, LHL, LHH, HLL, HLH, HHL, HHH], axis=1).

Sharding: pure data parallel over the 64 (b, c) slices; core m owns the 8
contiguous slices [8m, 8m+8).  Per core the kernel computes all 8 subbands
of its [8, 32, 128, 128] block.

On-core layout: partitions = (n, k) where n = local slice, k = d-pair index
(8 * 16 = 128 partitions).  Free dims hold (d-parity, h, w), so all three
Haar butterflies are elementwise ops along the free axis.

This walrus build gives most instruction encodings a SINGLE sync-wait slot
(2D DMAs get two), so the dataflow is arranged as a linear chain with at
most one cross-engine dependency per instruction:
  - DVE runs all three butterfly levels (plain tensor_add/sub),
  - ACT scales lD/hD by s^3 in place between the D and H levels,
  - loads ride the SP HWDGE queue, stores the ACT HWDGE queue.
The one wait Tile emits that cannot fit -- the load's write-after-write
wait on the DMA lane of the load two generations earlier -- is provably
implied by the load's other wait (the X-tile readers saw that DMA finish),
and is stripped by _prune_redundant_dma_waits below.
"""

import numpy as np

from concourse import bass, mybir
from concourse.bass_utils import run_bass_kernel_spmd
from concourse.tile import TileContext

_S3 = 2.0**-1.5  # (1/sqrt(2))**3 -- the three Haar levels' combined scale

N_CORES = 8
F32 = mybir.dt.float32
_SAME_ENGINE_RULE = True


def _prune_redundant_dma_waits(nc, verbose=False):
    """Drop DMAHW-semaphore waits from DMACopy instructions when they are
    transitively implied by the instruction's other waits.

    Tile's semaphore pass is per-proc minimal but not transitively minimal
    across procs: a reload of a tile slot waits both on the consumer engine
    (slot release) and on the DMA-lane semaphore of the slot's previous
    filler, even though the consumer's own data wait already implies the
    latter.  walrus' looped-DMA encoding has one sync-wait slot, so the
    redundant lane wait must go.

    Soundness: knowledge is tracked per proc (engine) in scheduled block
    order.  A proc learns (sem >= v) from its own waits, and importing
    through a wait on a single-producer engine semaphore merges the
    producer's knowledge snapshot at that increment (waits execute at the
    sequencer before the instruction, so program order carries knowledge).
    A DMAHW wait is dropped only if implied by that knowledge.
    """
    insts = []
    for b in nc.m.functions[0].blocks:
        insts.extend(b.instructions)

    # Identify single-producer pure-increment semaphores (per-engine sems).
    producers = {}  # sem id -> set of engines
    impure = set()  # sems with dec or register updates
    dma_incd = set()  # sems incremented by DMA completions (async wrt issue)
    for i in insts:
        si = i.sync_info
        for u in (si.on_update or []) if si else []:
            if (
                u.sync_type != "semaphore"
                or u.update_mode not in ("sem-inc", "sem-add-imm")
                or u.update_reg is not None
            ):
                impure.add(u.id)
            if str(i.opcode) == "DMACopy":
                dma_incd.add(u.id)
            producers.setdefault(u.id, set()).add(i.engine)
    single = {s for s, e in producers.items() if len(e) == 1 and s not in impure}

    know = {}  # engine -> {sem id: max known value}
    snaps = {}  # sem id -> list of (cum_value, knowledge dict copy)
    cum = {}  # sem id -> cumulative inc
    n_dropped = 0
    sem_engine = {s: next(iter(producers[s])) for s in single}
    # Engines whose ops complete in program order (DVE/ACT drain their
    # pipeline per op), so a wait on the engine's own semaphore at a value
    # already produced by earlier instructions is implied by program order.
    # GpSimd is excluded: its ops can overlap across the Q7 cluster (the
    # race detector models it as concurrent), so its waits are load-bearing.
    inorder = {mybir.EngineType.DVE, mybir.EngineType.Activation}

    def lookup(s, v):
        """Producer knowledge snapshot at the first increment reaching v."""
        for c, k in snaps.get(s, []):
            if c >= v:
                return k
        return None

    for i in insts:
        si = i.sync_info
        if si is None:
            continue
        waits = list(si.on_wait or [])
        k_eng = know.setdefault(i.engine, {})

        ok_waits = [
            w for w in waits
            if w.sync_type == "semaphore" and w.wait_mode == "sem-ge-imm" and w.wait_reg is None
        ]
        # Local view: proc knowledge + imports through single-producer waits.
        k_local = dict(k_eng)
        for w in ok_waits:
            if w.id in single:
                imp = lookup(w.id, w.wait_value)
                if imp:
                    for s, v in imp.items():
                        if k_local.get(s, -1) < v:
                            k_local[s] = v
            if k_local.get(w.id, -1) < w.wait_value:
                k_local[w.id] = w.wait_value

        op = str(i.opcode)
        if len(waits) > 1 and op != "EventSemaphore":
            ok_ids = {id(w) for w in ok_waits}
            # Rule (a): implied transitively through the instruction's other
            # waits?  (Mutual implication is impossible: a snapshot only
            # carries knowledge from causally-earlier events.)
            kept = []
            for w in waits:
                if (
                    id(w) in ok_ids
                    and _known_without(k_eng, ok_waits, w, single, lookup)
                    >= w.wait_value
                ):
                    n_dropped += 1
                else:
                    kept.append(w)
            # Rule (b), last resort to fit the single sync-wait slot: an
            # engine-executed op waiting on its own in-order engine's
            # semaphore at an already-produced value is implied by program
            # order.  (DMACopy excluded: the sequencer issues DMAs ahead of
            # engine completion.  GpSimd excluded: its ops can overlap.)
            # Applied only while >1 waits remain, because CoreSim's race
            # detector does not credit program order on instructions that
            # carry waits.
            if _SAME_ENGINE_RULE and len(kept) > 1:
                kept2 = []
                rest = len(kept)
                for w in kept:
                    if (
                        rest > 1
                        and id(w) in ok_ids
                        and op not in ("DMACopy", "Drain")
                        and w.id in single
                        and w.id not in dma_incd
                        and sem_engine[w.id] == i.engine
                        and i.engine in inorder
                        and cum.get(w.id, 0) >= w.wait_value
                    ):
                        n_dropped += 1
                        rest -= 1
                    else:
                        kept2.append(w)
                kept = kept2
            if len(kept) != len(waits):
                si.on_wait = kept
                i.sync_info = si
                if verbose:
                    print(f"pruned {i.name}: {[w.ant_name for w in waits]} -> "
                          f"{[w.ant_name for w in kept]}")

        # Proc knowledge advances by everything this instruction waited on
        # (kept or dropped -- dropped ones were implied anyway).
        for s, v in k_local.items():
            if k_eng.get(s, -1) < v:
                k_eng[s] = v

        for u in (si.on_update or []):
            if u.id in single:
                c = cum.get(u.id, 0) + u.update_value
                cum[u.id] = c
                snap = dict(k_eng)
                snap[u.id] = c
                snaps.setdefault(u.id, []).append((c, snap))

    return n_dropped


def _known_without(k_eng, ok_waits, w, single, lookup):
    """Max value of w's semaphore implied by proc knowledge plus imports
    through the OTHER single-producer waits of the same instruction."""
    best = k_eng.get(w.id, -1)
    for o in ok_waits:
        if o is w or o.id not in single:
            continue
        imp = lookup(o.id, o.wait_value)
        if imp and imp.get(w.id, -1) > best:
            best = imp.get(w.id, -1)
    return best


def build_dwt_nc(N=8, D=32, H=128, W=128, hc=16):
    """Bass program for one core: x [N, D, H, W] -> y [8, N, D/2, H/2, W/2].

    y's dim 0 is the subband in (d, h, w) binary order:
    LLL, LLH, LHL, LHH, HLL, HLH, HHL, HHH.
    `hc` = rows of H processed per pipeline iteration (must be even, divide H).
    """
    K = D // 2
    P = N * K
    assert P <= 128 and H % hc == 0 and hc % 2 == 0
    nc = bass.Bass()
    x = nc.dram_tensor("x", [N, D, H, W], F32, kind="ExternalInput")
    y = nc.dram_tensor("y", [8, N, K, H // 2, W // 2], F32, kind="ExternalOutput")

    # (n, k) merge into one partition dim; (j, i) merge into one contiguous
    # free run -- keeps every DMA access pattern within the 3-dim limit.
    xr = x[:].rearrange("n (k dp) h w -> (n k) dp h w", dp=2)
    # Partition-major destination view so one DMA stores all 8 subbands:
    # walk order (n k) then subband then (j i) matches the SBUF tile walk.
    yr = y[:].rearrange("s n k j i -> (n k) s (j i)")

    hw2 = (hc // 2) * (W // 2)  # output elements per partition per iteration

    last_Os = []
    with TileContext(nc) as tc:
        with tc.tile_pool(name="pool", bufs=2) as pool:
            for it in range(H // hc):
                h0 = it * hc
                X = pool.tile([P, 2, hc, W], F32, tag="X")
                # Loads go through gpsimd's SWDGE queue: its DMASW lane
                # semaphores are used by nothing else here, so loads get no
                # lane-serialization pre-wait and fit the single wait slot.
                nc.gpsimd.dma_start(out=X[:], in_=xr[:, :, h0 : h0 + hc, :])

                # D-level runs on GpSimd (fully contiguous operands) to take
                # ~a third of the butterfly off DVE, the bottleneck engine.
                lD = pool.tile([P, hc, W], F32, tag="lD")
                hD = pool.tile([P, hc, W], F32, tag="hD")
                nc.gpsimd.tensor_add(out=lD[:], in0=X[:, 0], in1=X[:, 1])
                nc.gpsimd.tensor_sub(out=hD[:], in0=X[:, 0], in1=X[:, 1])

                # All 8 subbands go into one packed tile so the iteration
                # needs a single store DMA (one DMA lane per iteration means
                # no lane-serialization pre-waits anywhere).
                O = pool.tile([P, 8, hc // 2, W // 2], F32, tag="O")
                last_Os = (last_Os + [O])[-2:]
                # Dedicated slot-recycle op: takes the previous store's
                # DMA-lane wait so the first real W-level op doesn't carry
                # two sync waits (its other hazards are credited through the
                # store lane's knowledge import).  The first two generations
                # use fresh slots (bufs=2) and need no recycle op.
                if it >= 2:
                    nc.vector.memset(O[0:1, 0, 0:1, 0:1], 0.0)
                for q, src in enumerate((lD, lD, hD, hD)):
                    hh = pool.tile([P, hc // 2, W], F32, tag=f"hh{q}")
                    ttop = nc.vector.tensor_add if q % 2 == 0 else nc.vector.tensor_sub
                    ttop(out=hh[:], in0=src[:, 0:hc:2, :], in1=src[:, 1:hc:2, :])

                    nc.vector.tensor_add(out=O[:, 2 * q], in0=hh[:, :, 0:W:2], in1=hh[:, :, 1:W:2])
                    nc.vector.tensor_sub(out=O[:, 2 * q + 1], in0=hh[:, :, 0:W:2], in1=hh[:, :, 1:W:2])

                # The s^3 scale rides on ACT after the W-level, off DVE's
                # critical path (one in-place pass over the packed tile).
                nc.scalar.mul(O[:], O[:], _S3)

                ji0 = it * hw2
                nc.scalar.dma_start(
                    out=yr[:, :, ji0 : ji0 + hw2],
                    in_=O[:].rearrange("p s j i -> p s (j i)"),
                )

                # GpSimd re-touches lD/hD (and reads X) after DVE's H-level,
                # becoming the last accessor of all three tiles: the next
                # generation's load then waits only on the Pool semaphore,
                # and the D-level's slot-release wait becomes transitively
                # implied by its load-lane wait (so it can be pruned in a
                # way the race detector also accepts).
                nc.gpsimd.tensor_copy(out=lD[0:1, 1:2, 0:1], in_=X[0:1, 0, 0:1, 0:1])
                nc.gpsimd.tensor_copy(out=hD[0:1, 1:2, 0:1], in_=X[0:1, 1, 0:1, 0:1])

            # Kernel-tail wait absorption (the tail Drain has one usable
            # sync-wait slot, so everything must funnel into DVE):
            #  - overwrite one element of the last two O tiles after their
            #    stores (absorbs those stores' DMA-lane semaphores),
            #  - read the element GpSimd's final re-touches wrote (absorbs
            #    the Pool semaphore's final value).
            for Ot in last_Os:
                nc.vector.memset(Ot[0:1, 0, 0:1, 0:1], 0.0)
            Z = pool.tile([1, 1], F32, tag="Ztail")
            nc.vector.tensor_copy(out=Z[:], in_=hD[0:1, 1:2, 0:1])

    _prune_redundant_dma_waits(nc)
    return nc


_NC_CACHE = {}


def _get_nc(hc=16):
    if hc not in _NC_CACHE:
        _NC_CACHE[hc] = build_dwt_nc(hc=hc)
    return _NC_CACHE[hc]


def _execute(x, trace=False, hc=16, **spmd_kwargs):
    """Run the SPMD kernel on the full input; returns ((LLL, H_all), results)."""
    x = np.asarray(x, dtype=np.float32)
    assert x.shape == (2, 32, 32, 128, 128), x.shape
    x64 = x.reshape(64, 32, 128, 128)
    in_maps = [{"x": x64[8 * m : 8 * m + 8]} for m in range(N_CORES)]

    nc = _get_nc(hc)
    res = run_bass_kernel_spmd(
        nc, in_maps, list(range(N_CORES)), trace=trace, **spmd_kwargs
    )

    # Per-core y: [8 sub, 8 n, 16, 64, 64]; flat (b,c) index = 8*m + n.
    arr = np.concatenate([res.results[m]["y"] for m in range(N_CORES)], axis=1)
    arr = arr.reshape(8, 2, 32, 16, 64, 64)
    LLL = np.ascontiguousarray(arr[0])
    H_all = np.ascontiguousarray(arr[1:].transpose(1, 0, 2, 3, 4, 5)).reshape(
        2, 224, 16, 64, 64
    )
    return (LLL, H_all), res


def kernel(x):
    return _execute(x)[0]


# revision 35
# speedup vs baseline: 1.2551x; 1.2551x over previous
"""3D Haar DWT (single level) on Trainium2, data-parallel over 8 NeuronCores.

Input  x: [2, 32, 32, 128, 128] f32  (B, C, D, H, W)
Output (LLL [2,32,16,64,64], H_all [2,224,16,64,64])  -- same pytree as the
reference: H_all = concat([LLH, LHL, LHH, HLL, HLH, HHL, HHH], axis=1).

Sharding: pure data parallel over the 64 (b, c) slices; core m owns the 8
contiguous slices [8m, 8m+8).  Per core the kernel computes all 8 subbands
of its [8, 32, 128, 128] block.

On-core layout: partitions = (n, k) where n = local slice, k = d-pair index
(8 * 16 = 128 partitions).  Free dims hold (d-parity, h, w), so all three
Haar butterflies are elementwise ops along the free axis.

This walrus build gives most instruction encodings a SINGLE sync-wait slot
(2D DMAs get two), so the dataflow is arranged as a linear chain with at
most one cross-engine dependency per instruction:
  - DVE runs all three butterfly levels (plain tensor_add/sub),
  - ACT scales lD/hD by s^3 in place between the D and H levels,
  - loads ride the SP HWDGE queue, stores the ACT HWDGE queue.
The one wait Tile emits that cannot fit -- the load's write-after-write
wait on the DMA lane of the load two generations earlier -- is provably
implied by the load's other wait (the X-tile readers saw that DMA finish),
and is stripped by _prune_redundant_dma_waits below.
"""

import numpy as np

from concourse import bass, mybir
from concourse.bass_utils import run_bass_kernel_spmd
from concourse.tile import TileContext

_S3 = 2.0**-1.5  # (1/sqrt(2))**3 -- the three Haar levels' combined scale

N_CORES = 8
F32 = mybir.dt.float32
_SAME_ENGINE_RULE = True


def _prune_redundant_dma_waits(nc, verbose=False):
    """Drop DMAHW-semaphore waits from DMACopy instructions when they are
    transitively implied by the instruction's other waits.

    Tile's semaphore pass is per-proc minimal but not transitively minimal
    across procs: a reload of a tile slot waits both on the consumer engine
    (slot release) and on the DMA-lane semaphore of the slot's previous
    filler, even though the consumer's own data wait already implies the
    latter.  walrus' looped-DMA encoding has one sync-wait slot, so the
    redundant lane wait must go.

    Soundness: knowledge is tracked per proc (engine) in scheduled block
    order.  A proc learns (sem >= v) from its own waits, and importing
    through a wait on a single-producer engine semaphore merges the
    producer's knowledge snapshot at that increment (waits execute at the
    sequencer before the instruction, so program order carries knowledge).
    A DMAHW wait is dropped only if implied by that knowledge.
    """
    insts = []
    for b in nc.m.functions[0].blocks:
        insts.extend(b.instructions)

    # Identify single-producer pure-increment semaphores (per-engine sems).
    producers = {}  # sem id -> set of engines
    impure = set()  # sems with dec or register updates
    dma_incd = set()  # sems incremented by DMA completions (async wrt issue)
    for i in insts:
        si = i.sync_info
        for u in (si.on_update or []) if si else []:
            if (
                u.sync_type != "semaphore"
                or u.update_mode not in ("sem-inc", "sem-add-imm")
                or u.update_reg is not None
            ):
                impure.add(u.id)
            if str(i.opcode) == "DMACopy":
                dma_incd.add(u.id)
            producers.setdefault(u.id, set()).add(i.engine)
    single = {s for s, e in producers.items() if len(e) == 1 and s not in impure}

    know = {}  # engine -> {sem id: max known value}
    snaps = {}  # sem id -> list of (cum_value, knowledge dict copy)
    cum = {}  # sem id -> cumulative inc
    n_dropped = 0
    sem_engine = {s: next(iter(producers[s])) for s in single}
    # Engines whose ops complete in program order (DVE/ACT drain their
    # pipeline per op), so a wait on the engine's own semaphore at a value
    # already produced by earlier instructions is implied by program order.
    # GpSimd is excluded: its ops can overlap across the Q7 cluster (the
    # race detector models it as concurrent), so its waits are load-bearing.
    inorder = {mybir.EngineType.DVE, mybir.EngineType.Activation}

    def lookup(s, v):
        """Producer knowledge snapshot at the first increment reaching v."""
        for c, k in snaps.get(s, []):
            if c >= v:
                return k
        return None

    for i in insts:
        si = i.sync_info
        if si is None:
            continue
        waits = list(si.on_wait or [])
        k_eng = know.setdefault(i.engine, {})

        ok_waits = [
            w for w in waits
            if w.sync_type == "semaphore" and w.wait_mode == "sem-ge-imm" and w.wait_reg is None
        ]
        # Local view: proc knowledge + imports through single-producer waits.
        k_local = dict(k_eng)
        for w in ok_waits:
            if w.id in single:
                imp = lookup(w.id, w.wait_value)
                if imp:
                    for s, v in imp.items():
                        if k_local.get(s, -1) < v:
                            k_local[s] = v
            if k_local.get(w.id, -1) < w.wait_value:
                k_local[w.id] = w.wait_value

        op = str(i.opcode)
        if len(waits) > 1 and op != "EventSemaphore":
            ok_ids = {id(w) for w in ok_waits}
            # Rule (a): implied transitively through the instruction's other
            # waits?  (Mutual implication is impossible: a snapshot only
            # carries knowledge from causally-earlier events.)
            kept = []
            for w in waits:
                if (
                    id(w) in ok_ids
                    and _known_without(k_eng, ok_waits, w, single, lookup)
                    >= w.wait_value
                ):
                    n_dropped += 1
                else:
                    kept.append(w)
            # Rule (b), last resort to fit the single sync-wait slot: an
            # engine-executed op waiting on its own in-order engine's
            # semaphore at an already-produced value is implied by program
            # order.  (DMACopy excluded: the sequencer issues DMAs ahead of
            # engine completion.  GpSimd excluded: its ops can overlap.)
            # Applied only while >1 waits remain, because CoreSim's race
            # detector does not credit program order on instructions that
            # carry waits.
            if _SAME_ENGINE_RULE and len(kept) > 1:
                kept2 = []
                rest = len(kept)
                for w in kept:
                    if (
                        rest > 1
                        and id(w) in ok_ids
                        and op not in ("DMACopy", "Drain")
                        and w.id in single
                        and w.id not in dma_incd
                        and sem_engine[w.id] == i.engine
                        and i.engine in inorder
                        and cum.get(w.id, 0) >= w.wait_value
                    ):
                        n_dropped += 1
                        rest -= 1
                    else:
                        kept2.append(w)
                kept = kept2
            if len(kept) != len(waits):
                si.on_wait = kept
                i.sync_info = si
                if verbose:
                    print(f"pruned {i.name}: {[w.ant_name for w in waits]} -> "
                          f"{[w.ant_name for w in kept]}")

        # Proc knowledge advances by everything this instruction waited on
        # (kept or dropped -- dropped ones were implied anyway).
        for s, v in k_local.items():
            if k_eng.get(s, -1) < v:
                k_eng[s] = v

        for u in (si.on_update or []):
            if u.id in single:
                c = cum.get(u.id, 0) + u.update_value
                cum[u.id] = c
                snap = dict(k_eng)
                snap[u.id] = c
                snaps.setdefault(u.id, []).append((c, snap))

    return n_dropped


def _known_without(k_eng, ok_waits, w, single, lookup):
    """Max value of w's semaphore implied by proc knowledge plus imports
    through the OTHER single-producer waits of the same instruction."""
    best = k_eng.get(w.id, -1)
    for o in ok_waits:
        if o is w or o.id not in single:
            continue
        imp = lookup(o.id, o.wait_value)
        if imp and imp.get(w.id, -1) > best:
            best = imp.get(w.id, -1)
    return best


def build_dwt_nc(N=8, D=32, H=128, W=128, hc=16):
    """Bass program for one core: x [N, D, H, W] -> y [8, N, D/2, H/2, W/2].

    y's dim 0 is the subband in (d, h, w) binary order:
    LLL, LLH, LHL, LHH, HLL, HLH, HHL, HHH.
    `hc` = rows of H processed per pipeline iteration (must be even, divide H).
    """
    K = D // 2
    P = N * K
    assert P <= 128 and H % hc == 0 and hc % 2 == 0
    nc = bass.Bass()
    x = nc.dram_tensor("x", [N, D, H, W], F32, kind="ExternalInput")
    y = nc.dram_tensor("y", [8, N, K, H // 2, W // 2], F32, kind="ExternalOutput")

    # (n, k) merge into one partition dim; (j, i) merge into one contiguous
    # free run -- keeps every DMA access pattern within the 3-dim limit.
    xr = x[:].rearrange("n (k dp) h w -> (n k) dp h w", dp=2)
    # Partition-major destination view so one DMA stores all 8 subbands:
    # walk order (n k) then subband then (j i) matches the SBUF tile walk.
    yr = y[:].rearrange("s n k j i -> (n k) s (j i)")

    hw2 = (hc // 2) * (W // 2)  # output elements per partition per iteration
    io_bufs = 3

    last_Os = []
    with TileContext(nc) as tc:
        with (
            tc.tile_pool(name="io", bufs=io_bufs) as io_pool,
            tc.tile_pool(name="work", bufs=2) as pool,
        ):
            for it in range(H // hc):
                h0 = it * hc
                X = io_pool.tile([P, 2, hc, W], F32, tag="X")
                # Loads go through gpsimd's SWDGE queue: its DMASW lane
                # semaphores are used by nothing else here, so loads get no
                # lane-serialization pre-wait and fit the single wait slot.
                # (GpSimd runs no compute in this kernel -- its TENSOR_TENSOR
                # throughput is ~0.5 elem/cycle AND its SBUF traffic slows
                # concurrent DVE ops by ~60%, measured on hardware.)
                nc.gpsimd.dma_start(out=X[:], in_=xr[:, :, h0 : h0 + hc, :])

                lD = pool.tile([P, hc, W], F32, tag="lD")
                hD = pool.tile([P, hc, W], F32, tag="hD")
                nc.vector.tensor_add(out=lD[:], in0=X[:, 0], in1=X[:, 1])
                nc.vector.tensor_sub(out=hD[:], in0=X[:, 0], in1=X[:, 1])

                # All 8 subbands go into one packed tile so the iteration
                # needs a single store DMA (one DMA lane per iteration means
                # no lane-serialization pre-waits anywhere).
                O = io_pool.tile([P, 8, hc // 2, W // 2], F32, tag="O")
                last_Os = (last_Os + [O])[-io_bufs:]
                # Dedicated slot-recycle op: takes the previous store's
                # DMA-lane wait so the first real W-level op doesn't carry
                # two sync waits (its other hazards are credited through the
                # store lane's knowledge import).  The first io_bufs
                # generations use fresh slots and need no recycle op.
                if it >= io_bufs:
                    nc.vector.memset(O[0:1, 0, 0:1, 0:1], 0.0)
                for q, src in enumerate((lD, lD, hD, hD)):
                    hh = pool.tile([P, hc // 2, W], F32, tag=f"hh{q}")
                    ttop = nc.vector.tensor_add if q % 2 == 0 else nc.vector.tensor_sub
                    ttop(out=hh[:], in0=src[:, 0:hc:2, :], in1=src[:, 1:hc:2, :])

                    nc.vector.tensor_add(out=O[:, 2 * q], in0=hh[:, :, 0:W:2], in1=hh[:, :, 1:W:2])
                    nc.vector.tensor_sub(out=O[:, 2 * q + 1], in0=hh[:, :, 0:W:2], in1=hh[:, :, 1:W:2])

                # The s^3 scale rides on ACT after the W-level, off DVE's
                # critical path (one in-place pass over the packed tile).
                nc.scalar.mul(O[:], O[:], _S3)

                ji0 = it * hw2
                nc.scalar.dma_start(
                    out=yr[:, :, ji0 : ji0 + hw2],
                    in_=O[:].rearrange("p s j i -> p s (j i)"),
                )

            # Kernel-tail wait absorption (the tail Drain has one usable
            # sync-wait slot, so everything must funnel into DVE): overwrite
            # one element of the last io_bufs O tiles after their stores,
            # absorbing those stores' DMA-lane semaphores into DVE.
            for Ot in last_Os:
                nc.vector.memset(Ot[0:1, 0, 0:1, 0:1], 0.0)

    _prune_redundant_dma_waits(nc)
    return nc


_NC_CACHE = {}


def _get_nc(hc=16):
    if hc not in _NC_CACHE:
        _NC_CACHE[hc] = build_dwt_nc(hc=hc)
    return _NC_CACHE[hc]


def _execute(x, trace=False, hc=16, **spmd_kwargs):
    """Run the SPMD kernel on the full input; returns ((LLL, H_all), results)."""
    x = np.asarray(x, dtype=np.float32)
    assert x.shape == (2, 32, 32, 128, 128), x.shape
    x64 = x.reshape(64, 32, 128, 128)
    in_maps = [{"x": x64[8 * m : 8 * m + 8]} for m in range(N_CORES)]

    nc = _get_nc(hc)
    res = run_bass_kernel_spmd(
        nc, in_maps, list(range(N_CORES)), trace=trace, **spmd_kwargs
    )

    # Per-core y: [8 sub, 8 n, 16, 64, 64]; flat (b,c) index = 8*m + n.
    arr = np.concatenate([res.results[m]["y"] for m in range(N_CORES)], axis=1)
    arr = arr.reshape(8, 2, 32, 16, 64, 64)
    LLL = np.ascontiguousarray(arr[0])
    H_all = np.ascontiguousarray(arr[1:].transpose(1, 0, 2, 3, 4, 5)).reshape(
        2, 224, 16, 64, 64
    )
    return (LLL, H_all), res


def kernel(x):
    return _execute(x)[0]


# revision 37
# speedup vs baseline: 1.3126x; 1.0458x over previous
"""3D Haar DWT (single level) on Trainium2, data-parallel over 8 NeuronCores.

Input  x: [2, 32, 32, 128, 128] f32  (B, C, D, H, W)
Output (LLL [2,32,16,64,64], H_all [2,224,16,64,64])  -- same pytree as the
reference: H_all = concat([LLH, LHL, LHH, HLL, HLH, HHL, HHH], axis=1).

Sharding: pure data parallel over the 64 (b, c) slices; core m owns the 8
contiguous slices [8m, 8m+8).  Per core the kernel computes all 8 subbands
of its [8, 32, 128, 128] block.

On-core layout: partitions = (n, k) where n = local slice, k = d-pair index
(8 * 16 = 128 partitions).  Free dims hold (d-parity, h, w), so all three
Haar butterflies are elementwise ops along the free axis.

This walrus build gives every instruction encoding a SINGLE sync-wait
slot, so the dataflow is arranged as a linear chain with at most one
cross-engine dependency per instruction:
  - DVE runs all three butterfly levels (plain tensor_add/sub); GpSimd
    compute was measured 2x slower AND contends with DVE for SBUF ports,
  - ACT scales the packed output tile by s^3 after the W-level, off DVE's
    critical path, and issues the stores on its HWDGE queue,
  - loads ride GpSimd's SWDGE queue (dedicated DMASW lane semaphores),
    one 2 MB load and one 2 MB packed store per iteration,
  - 1-element `memset` dummies absorb slot-recycle waits that would
    otherwise overflow an instruction's single wait slot, and
    _prune_redundant_dma_waits strips waits that are transitively implied
    (sound vector-clock reasoning that CoreSim's race detector agrees
    with).
"""

import numpy as np

from concourse import bass, mybir
from concourse.bass_utils import run_bass_kernel_spmd
from concourse.tile import TileContext

_S3 = 2.0**-1.5  # (1/sqrt(2))**3 -- the three Haar levels' combined scale

N_CORES = 8
F32 = mybir.dt.float32
_SAME_ENGINE_RULE = True


def _prune_redundant_dma_waits(nc, verbose=False):
    """Drop DMAHW-semaphore waits from DMACopy instructions when they are
    transitively implied by the instruction's other waits.

    Tile's semaphore pass is per-proc minimal but not transitively minimal
    across procs: a reload of a tile slot waits both on the consumer engine
    (slot release) and on the DMA-lane semaphore of the slot's previous
    filler, even though the consumer's own data wait already implies the
    latter.  walrus' looped-DMA encoding has one sync-wait slot, so the
    redundant lane wait must go.

    Soundness: knowledge is tracked per proc (engine) in scheduled block
    order.  A proc learns (sem >= v) from its own waits, and importing
    through a wait on a single-producer engine semaphore merges the
    producer's knowledge snapshot at that increment (waits execute at the
    sequencer before the instruction, so program order carries knowledge).
    A DMAHW wait is dropped only if implied by that knowledge.
    """
    insts = []
    for b in nc.m.functions[0].blocks:
        insts.extend(b.instructions)

    # Identify single-producer pure-increment semaphores (per-engine sems).
    producers = {}  # sem id -> set of engines
    impure = set()  # sems with dec or register updates
    dma_incd = set()  # sems incremented by DMA completions (async wrt issue)
    for i in insts:
        si = i.sync_info
        for u in (si.on_update or []) if si else []:
            if (
                u.sync_type != "semaphore"
                or u.update_mode not in ("sem-inc", "sem-add-imm")
                or u.update_reg is not None
            ):
                impure.add(u.id)
            if str(i.opcode) == "DMACopy":
                dma_incd.add(u.id)
            producers.setdefault(u.id, set()).add(i.engine)
    single = {s for s, e in producers.items() if len(e) == 1 and s not in impure}

    know = {}  # engine -> {sem id: max known value}
    snaps = {}  # sem id -> list of (cum_value, knowledge dict copy)
    cum = {}  # sem id -> cumulative inc
    n_dropped = 0
    sem_engine = {s: next(iter(producers[s])) for s in single}
    # Engines whose ops complete in program order (DVE/ACT drain their
    # pipeline per op), so a wait on the engine's own semaphore at a value
    # already produced by earlier instructions is implied by program order.
    # GpSimd is excluded: its ops can overlap across the Q7 cluster (the
    # race detector models it as concurrent), so its waits are load-bearing.
    inorder = {mybir.EngineType.DVE, mybir.EngineType.Activation}

    def lookup(s, v):
        """Producer knowledge snapshot at the first increment reaching v."""
        for c, k in snaps.get(s, []):
            if c >= v:
                return k
        return None

    for i in insts:
        si = i.sync_info
        if si is None:
            continue
        waits = list(si.on_wait or [])
        k_eng = know.setdefault(i.engine, {})

        ok_waits = [
            w for w in waits
            if w.sync_type == "semaphore" and w.wait_mode == "sem-ge-imm" and w.wait_reg is None
        ]
        # Local view: proc knowledge + imports through single-producer waits.
        k_local = dict(k_eng)
        for w in ok_waits:
            if w.id in single:
                imp = lookup(w.id, w.wait_value)
                if imp:
                    for s, v in imp.items():
                        if k_local.get(s, -1) < v:
                            k_local[s] = v
            if k_local.get(w.id, -1) < w.wait_value:
                k_local[w.id] = w.wait_value

        op = str(i.opcode)
        if len(waits) > 1 and op != "EventSemaphore":
            ok_ids = {id(w) for w in ok_waits}
            # Rule (a): implied transitively through the instruction's other
            # waits?  (Mutual implication is impossible: a snapshot only
            # carries knowledge from causally-earlier events.)
            kept = []
            for w in waits:
                if (
                    id(w) in ok_ids
                    and _known_without(k_eng, ok_waits, w, single, lookup)
                    >= w.wait_value
                ):
                    n_dropped += 1
                else:
                    kept.append(w)
            # Rule (b), last resort to fit the single sync-wait slot: an
            # engine-executed op waiting on its own in-order engine's
            # semaphore at an already-produced value is implied by program
            # order.  (DMACopy excluded: the sequencer issues DMAs ahead of
            # engine completion.  GpSimd excluded: its ops can overlap.)
            # Applied only while >1 waits remain, because CoreSim's race
            # detector does not credit program order on instructions that
            # carry waits.
            if _SAME_ENGINE_RULE and len(kept) > 1:
                kept2 = []
                rest = len(kept)
                for w in kept:
                    if (
                        rest > 1
                        and id(w) in ok_ids
                        and op not in ("DMACopy", "Drain")
                        and w.id in single
                        and w.id not in dma_incd
                        and sem_engine[w.id] == i.engine
                        and i.engine in inorder
                        and cum.get(w.id, 0) >= w.wait_value
                    ):
                        n_dropped += 1
                        rest -= 1
                    else:
                        kept2.append(w)
                kept = kept2
            if len(kept) != len(waits):
                si.on_wait = kept
                i.sync_info = si
                if verbose:
                    print(f"pruned {i.name}: {[w.ant_name for w in waits]} -> "
                          f"{[w.ant_name for w in kept]}")

        # Proc knowledge advances by everything this instruction waited on
        # (kept or dropped -- dropped ones were implied anyway).
        for s, v in k_local.items():
            if k_eng.get(s, -1) < v:
                k_eng[s] = v

        for u in (si.on_update or []):
            if u.id in single:
                c = cum.get(u.id, 0) + u.update_value
                cum[u.id] = c
                snap = dict(k_eng)
                snap[u.id] = c
                snaps.setdefault(u.id, []).append((c, snap))

    return n_dropped


def _known_without(k_eng, ok_waits, w, single, lookup):
    """Max value of w's semaphore implied by proc knowledge plus imports
    through the OTHER single-producer waits of the same instruction."""
    best = k_eng.get(w.id, -1)
    for o in ok_waits:
        if o is w or o.id not in single:
            continue
        imp = lookup(o.id, o.wait_value)
        if imp and imp.get(w.id, -1) > best:
            best = imp.get(w.id, -1)
    return best


def build_dwt_nc(N=8, D=32, H=128, W=128, hc=16):
    """Bass program for one core: x [N, D, H, W] -> y [8, N, D/2, H/2, W/2].

    y's dim 0 is the subband in (d, h, w) binary order:
    LLL, LLH, LHL, LHH, HLL, HLH, HHL, HHH.
    `hc` = rows of H processed per pipeline iteration (must be even, divide H).
    """
    K = D // 2
    P = N * K
    assert P <= 128 and H % hc == 0 and hc % 2 == 0
    nc = bass.Bass()
    x = nc.dram_tensor("x", [N, D, H, W], F32, kind="ExternalInput")
    y = nc.dram_tensor("y", [8, N, K, H // 2, W // 2], F32, kind="ExternalOutput")

    # (n, k) merge into one partition dim; (j, i) merge into one contiguous
    # free run -- keeps every DMA access pattern within the 3-dim limit.
    xr = x[:].rearrange("n (k dp) h w -> (n k) dp h w", dp=2)
    # Partition-major destination view so one DMA stores all 8 subbands:
    # walk order (n k) then subband then (j i) matches the SBUF tile walk.
    yr = y[:].rearrange("s n k j i -> (n k) s (j i)")

    io_bufs = 3
    # Small first/last chunks shorten the pipeline ramp: the first D-level
    # starts after a 1 MB load instead of 2 MB, and the final scale+store
    # tail is half-sized.
    chunks = [hc // 2] + [hc] * (H // hc - 1) + [hc // 2]

    last_Os = []
    h0 = 0
    with TileContext(nc) as tc:
        with (
            tc.tile_pool(name="io", bufs=io_bufs) as io_pool,
            tc.tile_pool(name="work", bufs=2) as pool,
        ):
            for it, ch in enumerate(chunks):
                X = io_pool.tile([P, 2, ch, W], F32, tag="X")
                # Loads go through gpsimd's SWDGE queue: its DMASW lane
                # semaphores are used by nothing else here, so loads get no
                # lane-serialization pre-wait and fit the single wait slot.
                # (GpSimd runs no compute in this kernel -- its TENSOR_TENSOR
                # throughput is ~0.5 elem/cycle AND its SBUF traffic slows
                # concurrent DVE ops by ~60%, measured on hardware.)
                nc.gpsimd.dma_start(out=X[:], in_=xr[:, :, h0 : h0 + ch, :])

                # lD and hD share one tile so the H-level's even/odd row
                # views merge across them: 2 ops instead of 4.
                DH = pool.tile([P, 2, ch, W], F32, tag="DH")
                nc.vector.tensor_add(out=DH[:, 0], in0=X[:, 0], in1=X[:, 1])
                nc.vector.tensor_sub(out=DH[:, 1], in0=X[:, 0], in1=X[:, 1])

                # All 8 subbands go into one packed tile so the iteration
                # needs a single store DMA (one DMA lane per iteration means
                # no lane-serialization pre-waits anywhere).
                O = io_pool.tile([P, 8, ch // 2, W // 2], F32, tag="O")
                last_Os = (last_Os + [O])[-io_bufs:]
                # Dedicated slot-recycle op: takes the previous store's
                # DMA-lane wait so the first real W-level op doesn't carry
                # two sync waits (its other hazards are credited through the
                # store lane's knowledge import).  The first io_bufs
                # generations use fresh slots and need no recycle op.
                if it >= io_bufs:
                    nc.vector.memset(O[0:1, 0, 0:1, 0:1], 0.0)

                # H-level: one add and one sub produce (lDlH, hDlH) and
                # (lDhH, hDhH) stacked on the free axis.  The (d-sel, even-h)
                # dims of DH merge into a single stride-256 view.
                ev = DH[:].rearrange("p d h w -> p (d h) w")[:, 0 : 2 * ch : 2, :]
                od = DH[:].rearrange("p d h w -> p (d h) w")[:, 1 : 2 * ch : 2, :]
                HHl = pool.tile([P, 2, ch // 2, W], F32, tag="HHl")
                HHh = pool.tile([P, 2, ch // 2, W], F32, tag="HHh")
                nc.vector.tensor_add(out=HHl[:].rearrange("p d h w -> p (d h) w"), in0=ev, in1=od)
                nc.vector.tensor_sub(out=HHh[:].rearrange("p d h w -> p (d h) w"), in0=ev, in1=od)

                # W-level: subband order is (d, h, w) binary, so HHl feeds
                # subbands (0, 1, 4, 5) and HHh (2, 3, 6, 7).
                for src, subs in ((HHl, (0, 4)), (HHh, (2, 6))):
                    for dsel in range(2):
                        s = subs[dsel]
                        nc.vector.tensor_add(
                            out=O[:, s], in0=src[:, dsel, :, 0:W:2], in1=src[:, dsel, :, 1:W:2]
                        )
                        nc.vector.tensor_sub(
                            out=O[:, s + 1], in0=src[:, dsel, :, 0:W:2], in1=src[:, dsel, :, 1:W:2]
                        )

                # The s^3 scale rides on ACT after the W-level, off DVE's
                # critical path (one in-place pass over the packed tile).
                nc.scalar.mul(O[:], O[:], _S3)

                ji0 = (h0 // 2) * (W // 2)
                nc.scalar.dma_start(
                    out=yr[:, :, ji0 : ji0 + (ch // 2) * (W // 2)],
                    in_=O[:].rearrange("p s j i -> p s (j i)"),
                )
                h0 += ch

            # Kernel-tail wait absorption (the tail Drain has one usable
            # sync-wait slot, so everything must funnel into DVE): overwrite
            # one element of the last io_bufs O tiles after their stores,
            # absorbing those stores' DMA-lane semaphores into DVE.
            for Ot in last_Os:
                nc.vector.memset(Ot[0:1, 0, 0:1, 0:1], 0.0)

    _prune_redundant_dma_waits(nc)
    return nc


_NC_CACHE = {}


def _get_nc(hc=16):
    if hc not in _NC_CACHE:
        _NC_CACHE[hc] = build_dwt_nc(hc=hc)
    return _NC_CACHE[hc]


def _execute(x, trace=False, hc=16, **spmd_kwargs):
    """Run the SPMD kernel on the full input; returns ((LLL, H_all), results)."""
    x = np.asarray(x, dtype=np.float32)
    assert x.shape == (2, 32, 32, 128, 128), x.shape
    x64 = x.reshape(64, 32, 128, 128)
    in_maps = [{"x": x64[8 * m : 8 * m + 8]} for m in range(N_CORES)]

    nc = _get_nc(hc)
    res = run_bass_kernel_spmd(
        nc, in_maps, list(range(N_CORES)), trace=trace, **spmd_kwargs
    )

    # Per-core y: [8 sub, 8 n, 16, 64, 64]; flat (b,c) index = 8*m + n.
    arr = np.concatenate([res.results[m]["y"] for m in range(N_CORES)], axis=1)
    arr = arr.reshape(8, 2, 32, 16, 64, 64)
    LLL = np.ascontiguousarray(arr[0])
    H_all = np.ascontiguousarray(arr[1:].transpose(1, 0, 2, 3, 4, 5)).reshape(
        2, 224, 16, 64, 64
    )
    return (LLL, H_all), res


def kernel(x):
    return _execute(x)[0]


# revision 38
# speedup vs baseline: 1.3168x; 1.0032x over previous
"""3D Haar DWT (single level) on Trainium2, data-parallel over 8 NeuronCores.

Input  x: [2, 32, 32, 128, 128] f32  (B, C, D, H, W)
Output (LLL [2,32,16,64,64], H_all [2,224,16,64,64])  -- same pytree as the
reference: H_all = concat([LLH, LHL, LHH, HLL, HLH, HHL, HHH], axis=1).

Sharding: pure data parallel over the 64 (b, c) slices; core m owns the 8
contiguous slices [8m, 8m+8).  Per core the kernel computes all 8 subbands
of its [8, 32, 128, 128] block.

On-core layout: partitions = (n, k) where n = local slice, k = d-pair index
(8 * 16 = 128 partitions).  Free dims hold (d-parity, h, w), so all three
Haar butterflies are elementwise ops along the free axis.

This walrus build gives every instruction encoding a SINGLE sync-wait
slot, so the dataflow is arranged as a linear chain with at most one
cross-engine dependency per instruction:
  - DVE runs all three butterfly levels (plain tensor_add/sub); GpSimd
    compute was measured 2x slower AND contends with DVE for SBUF ports,
  - ACT scales the packed output tile by s^3 after the W-level, off DVE's
    critical path, and issues the stores on its HWDGE queue,
  - loads ride GpSimd's SWDGE queue (dedicated DMASW lane semaphores),
    one 2 MB load and one 2 MB packed store per iteration,
  - 1-element `memset` dummies absorb slot-recycle waits that would
    otherwise overflow an instruction's single wait slot, and
    _prune_redundant_dma_waits strips waits that are transitively implied
    (sound vector-clock reasoning that CoreSim's race detector agrees
    with).
"""

import numpy as np

from concourse import bass, mybir
from concourse.bass_utils import run_bass_kernel_spmd
from concourse.tile import TileContext

_S3 = 2.0**-1.5  # (1/sqrt(2))**3 -- the three Haar levels' combined scale

N_CORES = 8
F32 = mybir.dt.float32
_SAME_ENGINE_RULE = True


def _prune_redundant_dma_waits(nc, verbose=False):
    """Drop DMAHW-semaphore waits from DMACopy instructions when they are
    transitively implied by the instruction's other waits.

    Tile's semaphore pass is per-proc minimal but not transitively minimal
    across procs: a reload of a tile slot waits both on the consumer engine
    (slot release) and on the DMA-lane semaphore of the slot's previous
    filler, even though the consumer's own data wait already implies the
    latter.  walrus' looped-DMA encoding has one sync-wait slot, so the
    redundant lane wait must go.

    Soundness: knowledge is tracked per proc (engine) in scheduled block
    order.  A proc learns (sem >= v) from its own waits, and importing
    through a wait on a single-producer engine semaphore merges the
    producer's knowledge snapshot at that increment (waits execute at the
    sequencer before the instruction, so program order carries knowledge).
    A DMAHW wait is dropped only if implied by that knowledge.
    """
    insts = []
    for b in nc.m.functions[0].blocks:
        insts.extend(b.instructions)

    # Identify single-producer pure-increment semaphores (per-engine sems).
    producers = {}  # sem id -> set of engines
    impure = set()  # sems with dec or register updates
    dma_incd = set()  # sems incremented by DMA completions (async wrt issue)
    for i in insts:
        si = i.sync_info
        for u in (si.on_update or []) if si else []:
            if (
                u.sync_type != "semaphore"
                or u.update_mode not in ("sem-inc", "sem-add-imm")
                or u.update_reg is not None
            ):
                impure.add(u.id)
            if str(i.opcode) == "DMACopy":
                dma_incd.add(u.id)
            producers.setdefault(u.id, set()).add(i.engine)
    single = {s for s, e in producers.items() if len(e) == 1 and s not in impure}

    know = {}  # engine -> {sem id: max known value}
    snaps = {}  # sem id -> list of (cum_value, knowledge dict copy)
    cum = {}  # sem id -> cumulative inc
    n_dropped = 0
    sem_engine = {s: next(iter(producers[s])) for s in single}
    # Engines whose ops complete in program order (DVE/ACT drain their
    # pipeline per op), so a wait on the engine's own semaphore at a value
    # already produced by earlier instructions is implied by program order.
    # GpSimd is excluded: its ops can overlap across the Q7 cluster (the
    # race detector models it as concurrent), so its waits are load-bearing.
    inorder = {mybir.EngineType.DVE, mybir.EngineType.Activation}

    def lookup(s, v):
        """Producer knowledge snapshot at the first increment reaching v."""
        for c, k in snaps.get(s, []):
            if c >= v:
                return k
        return None

    for i in insts:
        si = i.sync_info
        if si is None:
            continue
        waits = list(si.on_wait or [])
        k_eng = know.setdefault(i.engine, {})

        ok_waits = [
            w for w in waits
            if w.sync_type == "semaphore" and w.wait_mode == "sem-ge-imm" and w.wait_reg is None
        ]
        # Local view: proc knowledge + imports through single-producer waits.
        k_local = dict(k_eng)
        for w in ok_waits:
            if w.id in single:
                imp = lookup(w.id, w.wait_value)
                if imp:
                    for s, v in imp.items():
                        if k_local.get(s, -1) < v:
                            k_local[s] = v
            if k_local.get(w.id, -1) < w.wait_value:
                k_local[w.id] = w.wait_value

        op = str(i.opcode)
        if len(waits) > 1 and op != "EventSemaphore":
            ok_ids = {id(w) for w in ok_waits}
            # Rule (a): implied transitively through the instruction's other
            # waits?  (Mutual implication is impossible: a snapshot only
            # carries knowledge from causally-earlier events.)
            kept = []
            for w in waits:
                if (
                    id(w) in ok_ids
                    and _known_without(k_eng, ok_waits, w, single, lookup)
                    >= w.wait_value
                ):
                    n_dropped += 1
                else:
                    kept.append(w)
            # Rule (b), last resort to fit the single sync-wait slot: an
            # engine-executed op waiting on its own in-order engine's
            # semaphore at an already-produced value is implied by program
            # order.  (DMACopy excluded: the sequencer issues DMAs ahead of
            # engine completion.  GpSimd excluded: its ops can overlap.)
            # Applied only while >1 waits remain, because CoreSim's race
            # detector does not credit program order on instructions that
            # carry waits.
            if _SAME_ENGINE_RULE and len(kept) > 1:
                kept2 = []
                rest = len(kept)
                for w in kept:
                    if (
                        rest > 1
                        and id(w) in ok_ids
                        and op not in ("DMACopy", "Drain")
                        and w.id in single
                        and w.id not in dma_incd
                        and sem_engine[w.id] == i.engine
                        and i.engine in inorder
                        and cum.get(w.id, 0) >= w.wait_value
                    ):
                        n_dropped += 1
                        rest -= 1
                    else:
                        kept2.append(w)
                kept = kept2
            if len(kept) != len(waits):
                si.on_wait = kept
                i.sync_info = si
                if verbose:
                    print(f"pruned {i.name}: {[w.ant_name for w in waits]} -> "
                          f"{[w.ant_name for w in kept]}")

        # Proc knowledge advances by everything this instruction waited on
        # (kept or dropped -- dropped ones were implied anyway).
        for s, v in k_local.items():
            if k_eng.get(s, -1) < v:
                k_eng[s] = v

        for u in (si.on_update or []):
            if u.id in single:
                c = cum.get(u.id, 0) + u.update_value
                cum[u.id] = c
                snap = dict(k_eng)
                snap[u.id] = c
                snaps.setdefault(u.id, []).append((c, snap))

    return n_dropped


def _known_without(k_eng, ok_waits, w, single, lookup):
    """Max value of w's semaphore implied by proc knowledge plus imports
    through the OTHER single-producer waits of the same instruction."""
    best = k_eng.get(w.id, -1)
    for o in ok_waits:
        if o is w or o.id not in single:
            continue
        imp = lookup(o.id, o.wait_value)
        if imp and imp.get(w.id, -1) > best:
            best = imp.get(w.id, -1)
    return best


def build_dwt_nc(N=8, D=32, H=128, W=128, hc=16):
    """Bass program for one core: x [N, D, H, W] -> y [8, N, D/2, H/2, W/2].

    y's dim 0 is the subband in (d, h, w) binary order:
    LLL, LLH, LHL, LHH, HLL, HLH, HHL, HHH.
    `hc` = rows of H processed per pipeline iteration (must be even, divide H).
    """
    K = D // 2
    P = N * K
    assert P <= 128 and H % hc == 0 and hc % 2 == 0
    nc = bass.Bass()
    x = nc.dram_tensor("x", [N, D, H, W], F32, kind="ExternalInput")
    y = nc.dram_tensor("y", [8, N, K, H // 2, W // 2], F32, kind="ExternalOutput")

    # (n, k) merge into one partition dim; (j, i) merge into one contiguous
    # free run -- keeps every DMA access pattern within the 3-dim limit.
    xr = x[:].rearrange("n (k dp) h w -> (n k) dp h w", dp=2)
    # Partition-major destination view so one DMA stores all 8 subbands:
    # walk order (n k) then subband then (j i) matches the SBUF tile walk.
    yr = y[:].rearrange("s n k j i -> (n k) s (j i)")

    io_bufs = 3
    # Tapered chunks shorten the pipeline ramp at both ends: the first
    # D-level starts after a 0.5 MB load instead of 2 MB, and the final
    # scale+store tail shrinks likewise.  (More than 8 stores means store
    # DMA lanes repeat; the lane-serialization pre-waits that creates are
    # transitively implied via the scale -> memset knowledge chain and get
    # pruned.)
    if H // hc >= 4:
        q4, q2 = hc // 4, hc // 2
        chunks = [q4, q2 + q4] + [hc] * (H // hc - 2) + [q2 + q4, q4]
    else:
        chunks = [hc // 2] + [hc] * (H // hc - 1) + [hc // 2]
    assert sum(chunks) == H and all(c % 2 == 0 for c in chunks)

    last_Os = []
    h0 = 0
    with TileContext(nc) as tc:
        with (
            tc.tile_pool(name="io", bufs=io_bufs) as io_pool,
            tc.tile_pool(name="work", bufs=2) as pool,
        ):
            for it, ch in enumerate(chunks):
                X = io_pool.tile([P, 2, ch, W], F32, tag="X")
                # Loads go through gpsimd's SWDGE queue: its DMASW lane
                # semaphores are used by nothing else here, so loads get no
                # lane-serialization pre-wait and fit the single wait slot.
                # (GpSimd runs no compute in this kernel -- its TENSOR_TENSOR
                # throughput is ~0.5 elem/cycle AND its SBUF traffic slows
                # concurrent DVE ops by ~60%, measured on hardware.)
                nc.gpsimd.dma_start(out=X[:], in_=xr[:, :, h0 : h0 + ch, :])

                # lD and hD share one tile so the H-level's even/odd row
                # views merge across them: 2 ops instead of 4.
                DH = pool.tile([P, 2, ch, W], F32, tag="DH")
                nc.vector.tensor_add(out=DH[:, 0], in0=X[:, 0], in1=X[:, 1])
                nc.vector.tensor_sub(out=DH[:, 1], in0=X[:, 0], in1=X[:, 1])

                # All 8 subbands go into one packed tile so the iteration
                # needs a single store DMA (one DMA lane per iteration means
                # no lane-serialization pre-waits anywhere).
                O = io_pool.tile([P, 8, ch // 2, W // 2], F32, tag="O")
                last_Os = (last_Os + [O])[-io_bufs:]
                # Dedicated slot-recycle op: takes the previous store's
                # DMA-lane wait so the first real W-level op doesn't carry
                # two sync waits (its other hazards are credited through the
                # store lane's knowledge import).  The first io_bufs
                # generations use fresh slots and need no recycle op.
                if it >= io_bufs:
                    nc.vector.memset(O[0:1, 0, 0:1, 0:1], 0.0)

                # H-level: one add and one sub produce (lDlH, hDlH) and
                # (lDhH, hDhH) stacked on the free axis.  The (d-sel, even-h)
                # dims of DH merge into a single stride-256 view.
                ev = DH[:].rearrange("p d h w -> p (d h) w")[:, 0 : 2 * ch : 2, :]
                od = DH[:].rearrange("p d h w -> p (d h) w")[:, 1 : 2 * ch : 2, :]
                HHl = pool.tile([P, 2, ch // 2, W], F32, tag="HHl")
                HHh = pool.tile([P, 2, ch // 2, W], F32, tag="HHh")
                nc.vector.tensor_add(out=HHl[:].rearrange("p d h w -> p (d h) w"), in0=ev, in1=od)
                nc.vector.tensor_sub(out=HHh[:].rearrange("p d h w -> p (d h) w"), in0=ev, in1=od)

                # W-level: subband order is (d, h, w) binary, so HHl feeds
                # subbands (0, 1, 4, 5) and HHh (2, 3, 6, 7).
                for src, subs in ((HHl, (0, 4)), (HHh, (2, 6))):
                    for dsel in range(2):
                        s = subs[dsel]
                        nc.vector.tensor_add(
                            out=O[:, s], in0=src[:, dsel, :, 0:W:2], in1=src[:, dsel, :, 1:W:2]
                        )
                        nc.vector.tensor_sub(
                            out=O[:, s + 1], in0=src[:, dsel, :, 0:W:2], in1=src[:, dsel, :, 1:W:2]
                        )

                # The s^3 scale rides on ACT after the W-level, off DVE's
                # critical path (one in-place pass over the packed tile).
                nc.scalar.mul(O[:], O[:], _S3)

                ji0 = (h0 // 2) * (W // 2)
                nc.scalar.dma_start(
                    out=yr[:, :, ji0 : ji0 + (ch // 2) * (W // 2)],
                    in_=O[:].rearrange("p s j i -> p s (j i)"),
                )
                h0 += ch

            # Kernel-tail wait absorption (the tail Drain has one usable
            # sync-wait slot, so everything must funnel into DVE): overwrite
            # one element of the last io_bufs O tiles after their stores,
            # absorbing those stores' DMA-lane semaphores into DVE.
            for Ot in last_Os:
                nc.vector.memset(Ot[0:1, 0, 0:1, 0:1], 0.0)

    _prune_redundant_dma_waits(nc)
    return nc


_NC_CACHE = {}


def _get_nc(hc=16):
    if hc not in _NC_CACHE:
        _NC_CACHE[hc] = build_dwt_nc(hc=hc)
    return _NC_CACHE[hc]


def _execute(x, trace=False, hc=16, **spmd_kwargs):
    """Run the SPMD kernel on the full input; returns ((LLL, H_all), results)."""
    x = np.asarray(x, dtype=np.float32)
    assert x.shape == (2, 32, 32, 128, 128), x.shape
    x64 = x.reshape(64, 32, 128, 128)
    in_maps = [{"x": x64[8 * m : 8 * m + 8]} for m in range(N_CORES)]

    nc = _get_nc(hc)
    res = run_bass_kernel_spmd(
        nc, in_maps, list(range(N_CORES)), trace=trace, **spmd_kwargs
    )

    # Per-core y: [8 sub, 8 n, 16, 64, 64]; flat (b,c) index = 8*m + n.
    arr = np.concatenate([res.results[m]["y"] for m in range(N_CORES)], axis=1)
    arr = arr.reshape(8, 2, 32, 16, 64, 64)
    LLL = np.ascontiguousarray(arr[0])
    H_all = np.ascontiguousarray(arr[1:].transpose(1, 0, 2, 3, 4, 5)).reshape(
        2, 224, 16, 64, 64
    )
    return (LLL, H_all), res


def kernel(x):
    return _execute(x)[0]
